# revision 24
# baseline (speedup 1.0000x reference)
"""Trainium2 kernel for CompactBilinearLayer (count-sketch bilinear pooling).

Math: reference computes y = l2norm(signed_sqrt(sum_hw Re IFFT(FFT(x@M1)*FFT(x@M2)))).
Since M1/M2 are count-sketch matrices (one +-1 per row), FFT(x@M1) == x @ A1 with
A1[c,k] = s1[c] * exp(-2pi i h1[c] k / P) — a dense [512, K] matrix computable on the
host from M1 in O(C*K). The IFFT is linear, so the spatial sum moves before it.
Hermitian symmetry means only k = 0..4096 are needed; k=4096 has exactly-zero
imaginary parts (sin(pi*h)=0) and is handled by a tiny 1-partition side path plus a
rank-1 (-1)^s correction matmul into the IFFT accumulator.  Per core (4 batch
elements, 784 spatial positions — fully batch-local, no collectives):
  A: P projections = A^T @ x^T, single fp32r (RNE-11) matmul per tile — PE-bound
  B: S[k,b] = sum_t (P1*P2) per batch: Act stages PSUM->SBUF bf16, DVE does one
     broadcast-AP product op (2x rate), complex combine (D/E), one fold over t,
     then a reduce — bf16 2x where alignment allows
  C: IFFT via two-step factorization n=64q+s, batched every 4 freq tiles:
     Act broadcasts S over s, DVE builds U/V at bf16 2x, PE contracts over k%128
     with bf16 cosa/nsina stationaries into a persistent PSUM accumulator
  D: signed sqrt + per-batch L2 norm + store
"""
import numpy as np

P = 8192
C = 512
FT = 32            # frequency tiles of 128 -> k = 0..4095; k=4096 special-cased
NSLOT = FT * 128
NCORES = 8
BPC = 4            # batch elems per core
HW = 196           # spatial positions per batch elem
T = BPC * HW       # 784 positions per core
B = 32

# stage-C blocks: (first freq tile, number of tiles); small final blocks so the
# tail (last block's DVE prep -> PE -> stage D) is short
BLOCKS = [(0, 4), (4, 4), (8, 4), (12, 4), (16, 4), (20, 4), (24, 4),
          (28, 2), (30, 1), (31, 1)]

_CACHE = {}


def _build_program():
    import concourse.bass as bass
    import concourse.tile as tile
    from concourse import bacc, mybir

    f32 = mybir.dt.float32
    f32r = mybir.dt.float32r
    bf16 = mybir.dt.bfloat16
    nc = bacc.Bacc("TRN2", target_bir_lowering=False, debug=False,
                   num_devices=NCORES)

    ah_d = nc.dram_tensor("ah", [FT, C, 512], bf16, kind="ExternalInput").ap()
    xh_d = nc.dram_tensor("xh", [C, T], bf16, kind="ExternalInput").ap()
    b12_d = nc.dram_tensor("b12", [C, 2], bf16, kind="ExternalInput").ap()
    par_d = nc.dram_tensor("par", [1, 64], f32, kind="ExternalInput").ap()
    cphi_d = nc.dram_tensor("cphi", [FT, 128, 64], f32, kind="ExternalInput").ap()
    sphi_d = nc.dram_tensor("sphi", [FT, 128, 64], f32, kind="ExternalInput").ap()
    cosa_d = nc.dram_tensor("cosa", [128, 128], f32, kind="ExternalInput").ap()
    nsina_d = nc.dram_tensor("nsina", [128, 128], f32, kind="ExternalInput").ap()
    y_d = nc.dram_tensor("y", [BPC, P], f32, kind="ExternalOutput").ap()

    mult = mybir.AluOpType.mult
    Act = mybir.ActivationFunctionType

    with tile.TileContext(nc) as tc:
        with (
            tc.tile_pool(name="const", bufs=1) as const,
            tc.tile_pool(name="apool", bufs=4) as apool,
            tc.tile_pool(name="ps", bufs=3, space="PSUM") as pspool,
            tc.tile_pool(name="py", bufs=1, space="PSUM") as pypool,
            tc.tile_pool(name="scr", bufs=2) as scr,
            tc.tile_pool(name="uv", bufs=2) as uvpool,
        ):
            ah_tiles = {}

            def prefetch_ah(ft, split=False):
                if ft < FT and ft not in ah_tiles:
                    t_ = apool.tile([128, 4, 512], bf16, tag="ah",
                                    name=f"ah_{ft}")
                    if split:
                        for ck in range(4):
                            nc.sync.dma_start(
                                t_[:, ck],
                                ah_d[ft, ck * 128:(ck + 1) * 128, :])
                    else:
                        nc.sync.dma_start(
                            t_[:],
                            ah_d[ft].rearrange("(ck p) m -> p ck m", p=128))
                    ah_tiles[ft] = t_

            # x chunks + b12 first (they gate the k=4096 warm-up matmuls),
            # then the ah stream; stage-C tables ride the gpsimd DMA queue so
            # they never stall the ah prefetch pipeline
            xh_sb = const.tile([128, 4, T], bf16)
            b12_sb = const.tile([128, 4, 2], bf16)
            nc.sync.dma_start(b12_sb[:],
                              b12_d.rearrange("(ck p) m -> p ck m", p=128))
            for ck in range(4):
                nc.sync.dma_start(xh_sb[:, ck],
                                  xh_d[ck * 128:(ck + 1) * 128, :])
            prefetch_ah(0, split=True)
            prefetch_ah(1)
            prefetch_ah(2)
            prefetch_ah(3)

            par_sb = const.tile([1, 64], f32)
            nc.sync.dma_start(par_sb[:], par_d)
            cphi_sb = const.tile([128, FT, 64], f32)
            nc.sync.dma_start(cphi_sb[:], cphi_d.rearrange("kt p s -> p kt s"))
            sphi_sb = const.tile([128, FT, 64], f32)
            nc.sync.dma_start(sphi_sb[:], sphi_d.rearrange("kt p s -> p kt s"))
            cosa_sb = const.tile([128, 128], f32)
            nc.sync.dma_start(cosa_sb[:], cosa_d)
            nsina_sb = const.tile([128, 128], f32)
            nc.sync.dma_start(nsina_sb[:], nsina_d)
            ones_bf = const.tile([128, 1], bf16)
            nc.vector.memset(ones_bf[:], 1.0)
            onecol = const.tile([1, 128], bf16)
            nc.vector.memset(onecol[:], 1.0)

            # preload every activation table set used later (Sqrt lives in its
            # own set — loading it now keeps the ~1.3us table DMA off the tail)
            warm = const.tile([1, 1], f32)
            nc.vector.memset(warm[:], 1.0)
            for fn in (Act.Copy, Act.Abs, Act.Sign, Act.Sqrt):
                nc.scalar.activation(warm[:], warm[:], fn)

            # bf16 copies of the stage-C tables (DVE 2x mode needs bf16);
            # converted at the end of iteration 0 so their DMA wait never
            # blocks the hot Act copy FIFO
            cphib = const.tile([128, FT, 64], bf16)
            sphib = const.tile([128, FT, 64], bf16)
            cosab = const.tile([128, 128], bf16)
            nsinab = const.tile([128, 128], bf16)

            sresim = const.tile([128, FT, 2, 4], f32)
            psy = pypool.tile([128, BPC * 64], f32, tag="py")

            uu_tiles = {}
            vv_tiles = {}
            state = {"first_c": True}

            def emit_block_dve(bi):
                f0, nb = BLOCKS[bi]
                sx = scr.tile([128, nb, 2, 4, 64], bf16, tag=f"sx{nb}_{bi % 2}",
                              name=f"sx_{bi}")
                nc.scalar.activation(
                    sx[:],
                    sresim[:, f0:f0 + nb][:, :, :, :, None].broadcast_to(
                        [128, nb, 2, 4, 64]),
                    Act.Copy,
                )
                cph = cphib[:, f0:f0 + nb, None, :].broadcast_to(
                    [128, nb, 4, 64])
                sph = sphib[:, f0:f0 + nb, None, :].broadcast_to(
                    [128, nb, 4, 64])
                sre = sx[:, :, 0]
                sim = sx[:, :, 1]
                u1 = uvpool.tile([128, nb, 4, 64], bf16, tag=f"u1{nb}",
                                 name=f"u1_{bi}")
                u2 = uvpool.tile([128, nb, 4, 64], bf16, tag=f"u2{nb}",
                                 name=f"u2_{bi}")
                uu = uvpool.tile([128, nb, 4, 64], bf16, tag=f"uu{nb}",
                                 name=f"uu_{bi}")
                v1 = uvpool.tile([128, nb, 4, 64], bf16, tag=f"v1{nb}",
                                 name=f"v1_{bi}")
                v2 = uvpool.tile([128, nb, 4, 64], bf16, tag=f"v2{nb}",
                                 name=f"v2_{bi}")
                vv = uvpool.tile([128, nb, 4, 64], bf16, tag=f"vv{nb}",
                                 name=f"vv_{bi}")
                nc.vector.tensor_tensor(u1[:], cph, sre, op=mult)
                nc.vector.tensor_tensor(u2[:], sph, sim, op=mult)
                nc.vector.tensor_sub(uu[:], u1[:], u2[:])
                nc.vector.tensor_tensor(v1[:], sph, sre, op=mult)
                nc.vector.tensor_tensor(v2[:], cph, sim, op=mult)
                nc.vector.tensor_add(vv[:], v1[:], v2[:])
                uu_tiles[bi] = uu
                vv_tiles[bi] = vv

            def emit_block_pe(bi):
                f0, nb = BLOCKS[bi]
                uu, vv = uu_tiles[bi], vv_tiles[bi]
                for j in range(nb):
                    nc.tensor.matmul(psy[:], cosab[:], uu[:, j],
                                     start=state["first_c"], stop=False)
                    state["first_c"] = False
                for j in range(nb):
                    nc.tensor.matmul(psy[:], nsinab[:], vv[:, j],
                                     start=False, stop=False)

            pe_done = set()

            # k=4096 side path first: its matmuls only need x+b12, so they
            # fill the window while the first ah tile is still streaming in
            psx1 = pspool.tile([128, T], f32, tag="ps", name="psx1")
            psx2 = pspool.tile([128, T], f32, tag="ps", name="psx2")
            for ck in range(4):
                for psx, mi in ((psx1, 0), (psx2, 1)):
                    for c0, cn in ((0, 512), (512, T - 512)):
                        nc.tensor.matmul(
                            psx[0:1, c0:c0 + cn],
                            b12_sb[:, ck, mi:mi + 1],
                            xh_sb[:, ck, c0:c0 + cn],
                            start=(ck == 0),
                            stop=(ck == 3),
                        )
            sp1 = scr.tile([1, T], f32, tag="sp1")
            nc.scalar.activation(sp1[:], psx1[0:1, :], Act.Copy)
            qs = scr.tile([1, T], f32, tag="qs")
            nc.vector.tensor_tensor(qs[:], psx2[0:1, :], sp1[:], op=mult)
            f1s = scr.tile([1, 4, 98], f32, tag="f1s")
            qsv = qs[:].rearrange("p (b h t) -> p b h t", b=BPC, h=2)
            nc.vector.tensor_add(f1s[:], qsv[:, :, 0], qsv[:, :, 1])
            s4 = scr.tile([1, 4], f32, tag="s4")
            nc.vector.reduce_sum(out=s4[:], in_=f1s[:],
                                 axis=mybir.AxisListType.X)
            r4 = scr.tile([1, 4, 64], bf16, tag="r4")
            nc.vector.tensor_tensor(
                r4[:],
                s4[:][:, :, None].broadcast_to([1, 4, 64]),
                par_sb[:][:, None, :].broadcast_to([1, 4, 64]),
                op=mult,
            )

            # ---- main loop over frequency tiles ----
            for ft in range(FT):
                # stage-C DVE prep for the block ending at ft-1 (one-ft lag so
                # its sem wait never blocks the Act copy FIFO)
                for bi, (f0, nb) in enumerate(BLOCKS):
                    if f0 + nb == ft:
                        emit_block_dve(bi)
                for bi, (f0, nb) in enumerate(BLOCKS):
                    if bi not in pe_done and ft >= f0 + nb + 2:
                        emit_block_pe(bi)
                        pe_done.add(bi)

                prefetch_ah(ft + 3)
                ah_t = ah_tiles.pop(ft)

                # stage A: projections, one Act copy per m to SBUF bf16
                pc01 = scr.tile([128, 2, T], bf16, tag="pc01",
                                name=f"pc01_{ft}")
                pc23 = scr.tile([128, 2, T], bf16, tag="pc23",
                                name=f"pc23_{ft}")
                pcs = (pc01[:, 0], pc01[:, 1], pc23[:, 0], pc23[:, 1])
                for m in range(4):
                    ps_m = pspool.tile([128, T], f32, tag="ps",
                                       name=f"ps{m}_{ft}")
                    msl = slice(m * 128, (m + 1) * 128)
                    for c0, cn in ((0, 512), (512, T - 512)):
                        for ck in range(4):
                            nc.tensor.matmul(
                                ps_m[:, c0:c0 + cn],
                                ah_t[:, ck, msl],
                                xh_sb[:, ck, c0:c0 + cn],
                                start=(ck == 0),
                                stop=(ck == 3),
                            )
                    nc.scalar.activation(pcs[m], ps_m[:], Act.Copy)

                # stage B: products (one broadcast-AP op), complex combine,
                # one fold over t, reduce
                q = scr.tile([128, 2, 2, T], bf16, tag="q", name=f"q_{ft}")
                nc.vector.tensor_tensor(
                    q[:],
                    pc01[:, :, None, :].broadcast_to([128, 2, 2, T]),
                    pc23[:, None, :, :].broadcast_to([128, 2, 2, T]),
                    op=mult,
                )
                # complex combine on GpSimd (otherwise idle) to unload the
                # 91%-busy DVE; last tiles stay on DVE to keep tail latency low
                de = scr.tile([128, 2, T], bf16, tag="de", name=f"de_{ft}")
                eng = nc.gpsimd if ft < FT - 3 else nc.vector
                eng.tensor_sub(de[:, 0], q[:, 0, 0], q[:, 1, 1])
                eng.tensor_add(de[:, 1], q[:, 0, 1], q[:, 1, 0])
                f1 = scr.tile([128, 2, 4, 98], bf16, tag="f1",
                              name=f"f1_{ft}")
                dev = de[:].rearrange("p e (b h t) -> p e b h t", b=BPC, h=2)
                nc.vector.tensor_add(f1[:], dev[:, :, :, 0], dev[:, :, :, 1])
                nc.vector.reduce_sum(
                    out=sresim[:, ft],
                    in_=f1[:],
                    axis=mybir.AxisListType.X,
                )

                if ft == 2:
                    nc.scalar.activation(cphib[:], cphi_sb[:], Act.Copy)
                    nc.scalar.activation(sphib[:], sphi_sb[:], Act.Copy)
                    nc.scalar.activation(cosab[:], cosa_sb[:], Act.Copy)
                    nc.scalar.activation(nsinab[:], nsina_sb[:], Act.Copy)

            for bi, (f0, nb) in enumerate(BLOCKS):
                if f0 + nb >= FT:
                    emit_block_dve(bi)
            for bi in range(len(BLOCKS)):
                if bi not in pe_done:
                    emit_block_pe(bi)
                    pe_done.add(bi)

            # rank-1 k=4096 correction: psy[q, b, s] += 1 * (S4096[b]*par[s]);
            # closes the psy accumulation group
            nc.tensor.matmul(psy[:], onecol[0:1, :], r4[:],
                             start=False, stop=True)

            # ---- stage D: signed sqrt, per-batch l2 norm, store ----
            absy = scr.tile([128, BPC * 64], bf16, tag="absy")
            nc.scalar.activation(absy[:], psy[:], Act.Abs)
            psn = pspool.tile([128, T], f32, tag="ps", name="psn")
            nc.tensor.matmul(psn[0:1, 0:BPC * 64], ones_bf[:], absy[:],
                             start=True, stop=True)
            sqy = scr.tile([128, BPC * 64], bf16, tag="sqy")
            nc.scalar.activation(sqy[:], absy[:], Act.Sqrt)
            sgn = scr.tile([128, BPC * 64], bf16, tag="sgn")
            nc.scalar.activation(sgn[:], psy[:], Act.Sign)
            nsq = scr.tile([1, BPC], f32, tag="nsq")
            nc.vector.reduce_sum(
                out=nsq[:],
                in_=psn[0:1, 0:BPC * 64].rearrange("p (b s) -> p b s", b=BPC),
                axis=mybir.AxisListType.X,
            )
            nc.vector.tensor_scalar_max(nsq[:], nsq[:], 1e-10)
            ys = scr.tile([128, BPC * 64], bf16, tag="ys")
            nc.vector.tensor_mul(ys[:], sqy[:], sgn[:])
            sqn = scr.tile([1, BPC], f32, tag="sqn")
            nc.scalar.activation(sqn[:], nsq[:], Act.Sqrt)
            invn = scr.tile([1, BPC], f32, tag="invn")
            nc.vector.reciprocal(invn[:], sqn[:])

            onesrow = const.tile([1, 128], f32)
            nc.vector.memset(onesrow[:], 1.0)
            psb = pspool.tile([128, T], f32, tag="ps", name="psb")
            nc.tensor.matmul(psb[:, 0:BPC], onesrow[0:1, :], invn[0:1, :],
                             start=True, stop=True)
            inv_b = psb[:, 0:BPC][:, :, None].broadcast_to([128, BPC, 64])
            fin = scr.tile([128, BPC * 64], f32, tag="fin")
            nc.vector.tensor_tensor(
                fin[:].rearrange("p (b s) -> p b s", b=BPC),
                ys[:].rearrange("p (b s) -> p b s", b=BPC),
                inv_b,
                op=mult,
            )
            nc.sync.dma_start(
                y_d.rearrange("b (q s) -> q b s", q=128),
                fin[:].rearrange("p (b s) -> p b s", b=BPC),
            )

    nc.compile()
    return nc


def _round_fp32r(f):
    """RNE to 11 mantissa bits — matches TRN2 fp32r rounding exactly."""
    u = np.ascontiguousarray(f).view(np.uint32)
    drop = 12
    r = u + np.uint32((1 << (drop - 1)) - 1) + ((u >> drop) & np.uint32(1))
    r = (r >> drop) << drop
    return r.view(np.float32)


def _host_prep(x, M1, M2):
    x = np.ascontiguousarray(np.asarray(x, np.float32))
    M1 = np.asarray(M1, np.float32)
    M2 = np.asarray(M2, np.float32)

    h1 = np.argmax(np.abs(M1), axis=1)
    s1 = M1[np.arange(C), h1].astype(np.float64)
    h2 = np.argmax(np.abs(M2), axis=1)
    s2 = M2[np.arange(C), h2].astype(np.float64)

    k = np.arange(NSLOT, dtype=np.float64)
    ang1 = 2 * np.pi * np.outer(h1.astype(np.float64), k) / P
    ang2 = 2 * np.pi * np.outer(h2.astype(np.float64), k) / P
    # a[ft, c, m*128 + j]: m in (A1re, A1im, A2re, A2im), freq = ft*128 + j
    a = np.empty((FT, C, 512), np.float32)
    a1re = (s1[:, None] * np.cos(ang1)).astype(np.float32)
    a1im = (-s1[:, None] * np.sin(ang1)).astype(np.float32)
    a2re = (s2[:, None] * np.cos(ang2)).astype(np.float32)
    a2im = (-s2[:, None] * np.sin(ang2)).astype(np.float32)
    for ft in range(FT):
        ksl = slice(ft * 128, (ft + 1) * 128)
        a[ft, :, 0:128] = a1re[:, ksl]
        a[ft, :, 128:256] = a1im[:, ksl]
        a[ft, :, 256:384] = a2re[:, ksl]
        a[ft, :, 384:512] = a2im[:, ksl]

    # k = 4096: A[c] = s * cos(pi*h) = s * (-1)^h (imag part exactly 0)
    b12 = np.stack([
        (s1 * np.cos(np.pi * h1.astype(np.float64))).astype(np.float32),
        (s2 * np.cos(np.pi * h2.astype(np.float64))).astype(np.float32),
    ], axis=1)
    # y[64q+s] += (1/P) * S4096 * (-1)^s
    par = ((1.0 / P) * np.cos(np.pi * np.arange(64, dtype=np.float64))
           ).astype(np.float32).reshape(1, 64)

    w = np.full(NSLOT, 2.0 / P)
    w[0] = 1.0 / P
    s_idx = np.arange(64, dtype=np.float64)
    phi = 2 * np.pi * np.outer(k, s_idx) / P
    cphi = (w[:, None] * np.cos(phi)).astype(np.float32).reshape(FT, 128, 64)
    sphi = (w[:, None] * np.sin(phi)).astype(np.float32).reshape(FT, 128, 64)

    km = np.arange(128, dtype=np.float64)
    alpha = 2 * np.pi * np.outer(km, km) / 128
    cosa = np.cos(alpha).astype(np.float32)
    nsina = (-np.sin(alpha)).astype(np.float32)

    import ml_dtypes

    xt = np.ascontiguousarray(x.reshape(B * HW, C).T)  # [C, 6272]

    ah = a.astype(ml_dtypes.bfloat16)
    xh = xt.astype(ml_dtypes.bfloat16)
    b12 = b12.astype(ml_dtypes.bfloat16)
    return ah, b12, par, cphi, sphi, cosa, nsina, xh


def _make_in_maps(x, M1, M2):
    ah, b12, par, cphi, sphi, cosa, nsina, xh = _host_prep(x, M1, M2)
    in_maps = []
    for r in range(NCORES):
        in_maps.append({
            "ah": ah,
            "xh": np.ascontiguousarray(xh[:, r * T:(r + 1) * T]),
            "b12": b12,
            "par": par,
            "cphi": cphi,
            "sphi": sphi,
            "cosa": cosa,
            "nsina": nsina,
        })
    return in_maps


def kernel(x, M1, M2):
    from concourse.bass_utils import run_bass_kernel_spmd

    if "nc" not in _CACHE:
        _CACHE["nc"] = _build_program()
    nc = _CACHE["nc"]

    in_maps = _make_in_maps(x, M1, M2)
    res = run_bass_kernel_spmd(nc, in_maps, core_ids=list(range(NCORES)))
    out = np.concatenate([res.results[r]["y"] for r in range(NCORES)], axis=0)
    return out.astype(np.float32)


# revision 25
# speedup vs baseline: 1.2683x; 1.2683x over previous
"""Trainium2 kernel for CompactBilinearLayer (count-sketch bilinear pooling).

Math: reference computes y = l2norm(signed_sqrt(sum_hw Re IFFT(FFT(x@M1)*FFT(x@M2)))).
Since M1/M2 are count-sketch matrices (one +-1 per row), FFT(x@M1) == x @ A1 with
A1[c,k] = s1[c] * exp(-2pi i h1[c] k / P) — a dense [512, K] matrix computable on the
host from M1 in O(C*K). The IFFT is linear, so the spatial sum moves before it.
Hermitian symmetry means only k = 0..4096 are needed; k=4096 has exactly-zero
imaginary parts (sin(pi*h)=0) and is handled by a tiny 1-partition side path plus a
rank-1 (-1)^s correction matmul into the IFFT accumulator.  Per core (4 batch
elements, 784 spatial positions — fully batch-local, no collectives):
  A: P projections = A^T @ x^T, single fp32r (RNE-11) matmul per tile — PE-bound
  B: S[k,b] = sum_t (P1*P2) per batch: Act stages PSUM->SBUF bf16, DVE does one
     broadcast-AP product op (2x rate), complex combine (D/E), one fold over t,
     then a reduce — bf16 2x where alignment allows
  C: IFFT via two-step factorization n=64q+s, batched every 4 freq tiles:
     Act broadcasts S over s, DVE builds U/V at bf16 2x, PE contracts over k%128
     with bf16 cosa/nsina stationaries into a persistent PSUM accumulator
  D: signed sqrt + per-batch L2 norm + store
"""
import numpy as np

P = 8192
C = 512
FT = 32            # frequency tiles of 128 -> k = 0..4095; k=4096 special-cased
NSLOT = FT * 128
NCORES = 8
BPC = 4            # batch elems per core
HW = 196           # spatial positions per batch elem
T = BPC * HW       # 784 positions per core
B = 32

# stage-C blocks: (first freq tile, number of tiles); small final blocks so the
# tail (last block's DVE prep -> PE -> stage D) is short
BLOCKS = [(0, 4), (4, 4), (8, 4), (12, 4), (16, 4), (20, 4), (24, 4),
          (28, 2), (30, 1), (31, 1)]

_CACHE = {}


def _build_program():
    import concourse.bass as bass
    import concourse.tile as tile
    from concourse import bacc, mybir

    f32 = mybir.dt.float32
    f32r = mybir.dt.float32r
    bf16 = mybir.dt.bfloat16
    nc = bacc.Bacc("TRN2", target_bir_lowering=False, debug=False,
                   num_devices=NCORES)

    ah_d = nc.dram_tensor("ah", [FT, C, 512], bf16, kind="ExternalInput").ap()
    xh_d = nc.dram_tensor("xh", [C, T], bf16, kind="ExternalInput").ap()
    b12_d = nc.dram_tensor("b12", [C, 2], bf16, kind="ExternalInput").ap()
    par_d = nc.dram_tensor("par", [1, 64], f32, kind="ExternalInput").ap()
    cphi_d = nc.dram_tensor("cphi", [FT, 128, 64], f32, kind="ExternalInput").ap()
    sphi_d = nc.dram_tensor("sphi", [FT, 128, 64], f32, kind="ExternalInput").ap()
    cosa_d = nc.dram_tensor("cosa", [128, 128], f32, kind="ExternalInput").ap()
    nsina_d = nc.dram_tensor("nsina", [128, 128], f32, kind="ExternalInput").ap()
    y_d = nc.dram_tensor("y", [BPC, P], f32, kind="ExternalOutput").ap()

    mult = mybir.AluOpType.mult
    Act = mybir.ActivationFunctionType

    with tile.TileContext(nc) as tc:
        with (
            tc.tile_pool(name="const", bufs=1) as const,
            tc.tile_pool(name="apool", bufs=4) as apool,
            tc.tile_pool(name="ps", bufs=3, space="PSUM") as pspool,
            tc.tile_pool(name="py", bufs=1, space="PSUM") as pypool,
            tc.tile_pool(name="scr", bufs=2) as scr,
            tc.tile_pool(name="uv", bufs=2) as uvpool,
        ):
            ah_tiles = {}

            def prefetch_ah(ft, split=False):
                if ft < FT and ft not in ah_tiles:
                    t_ = apool.tile([128, 4, 512], bf16, tag="ah",
                                    name=f"ah_{ft}")
                    if split:
                        for ck in range(4):
                            nc.sync.dma_start(
                                t_[:, ck],
                                ah_d[ft, ck * 128:(ck + 1) * 128, :])
                    else:
                        nc.sync.dma_start(
                            t_[:],
                            ah_d[ft].rearrange("(ck p) m -> p ck m", p=128))
                    ah_tiles[ft] = t_

            # x chunks + b12 first (they gate the k=4096 warm-up matmuls),
            # then the ah stream; stage-C tables ride the gpsimd DMA queue so
            # they never stall the ah prefetch pipeline
            xh_sb = const.tile([128, 4, T], bf16)
            b12_sb = const.tile([128, 4, 2], bf16)
            nc.sync.dma_start(b12_sb[:],
                              b12_d.rearrange("(ck p) m -> p ck m", p=128))
            for ck in range(4):
                nc.sync.dma_start(xh_sb[:, ck],
                                  xh_d[ck * 128:(ck + 1) * 128, :])
            prefetch_ah(0, split=True)
            prefetch_ah(1)
            prefetch_ah(2)
            prefetch_ah(3)

            par_sb = const.tile([1, 64], f32)
            nc.sync.dma_start(par_sb[:], par_d)
            cphi_sb = const.tile([128, FT, 64], f32)
            nc.sync.dma_start(cphi_sb[:], cphi_d.rearrange("kt p s -> p kt s"))
            sphi_sb = const.tile([128, FT, 64], f32)
            nc.sync.dma_start(sphi_sb[:], sphi_d.rearrange("kt p s -> p kt s"))
            cosa_sb = const.tile([128, 128], f32)
            nc.sync.dma_start(cosa_sb[:], cosa_d)
            nsina_sb = const.tile([128, 128], f32)
            nc.sync.dma_start(nsina_sb[:], nsina_d)
            ones_bf = const.tile([128, 1], bf16)
            nc.vector.memset(ones_bf[:], 1.0)
            onecol = const.tile([1, 128], bf16)
            nc.vector.memset(onecol[:], 1.0)

            # preload every activation table set used later (Sqrt lives in its
            # own set — loading it now keeps the ~1.3us table DMA off the tail)
            warm = const.tile([1, 1], f32)
            nc.vector.memset(warm[:], 1.0)
            for fn in (Act.Copy, Act.Abs, Act.Sign, Act.Sqrt):
                nc.scalar.activation(warm[:], warm[:], fn)

            # bf16 copies of the stage-C tables (DVE 2x mode needs bf16);
            # converted at the end of iteration 0 so their DMA wait never
            # blocks the hot Act copy FIFO
            cphib = const.tile([128, FT, 64], bf16)
            sphib = const.tile([128, FT, 64], bf16)
            cosab = const.tile([128, 128], bf16)
            nsinab = const.tile([128, 128], bf16)

            sresim = const.tile([128, FT, 2, 4], f32)
            psy = pypool.tile([128, BPC * 64], f32, tag="py")

            uu_tiles = {}
            vv_tiles = {}
            state = {"first_c": True}

            def emit_block_dve(bi):
                f0, nb = BLOCKS[bi]
                sx = scr.tile([128, nb, 2, 4, 64], bf16, tag=f"sx{nb}_{bi % 2}",
                              name=f"sx_{bi}")
                nc.scalar.activation(
                    sx[:],
                    sresim[:, f0:f0 + nb][:, :, :, :, None].broadcast_to(
                        [128, nb, 2, 4, 64]),
                    Act.Copy,
                )
                cph = cphib[:, f0:f0 + nb, None, :].broadcast_to(
                    [128, nb, 4, 64])
                sph = sphib[:, f0:f0 + nb, None, :].broadcast_to(
                    [128, nb, 4, 64])
                sre = sx[:, :, 0]
                sim = sx[:, :, 1]
                u1 = uvpool.tile([128, nb, 4, 64], bf16, tag=f"u1{nb}",
                                 name=f"u1_{bi}")
                u2 = uvpool.tile([128, nb, 4, 64], bf16, tag=f"u2{nb}",
                                 name=f"u2_{bi}")
                uu = uvpool.tile([128, nb, 4, 64], bf16, tag=f"uu{nb}",
                                 name=f"uu_{bi}")
                v1 = uvpool.tile([128, nb, 4, 64], bf16, tag=f"v1{nb}",
                                 name=f"v1_{bi}")
                v2 = uvpool.tile([128, nb, 4, 64], bf16, tag=f"v2{nb}",
                                 name=f"v2_{bi}")
                vv = uvpool.tile([128, nb, 4, 64], bf16, tag=f"vv{nb}",
                                 name=f"vv_{bi}")
                nc.vector.tensor_tensor(u1[:], cph, sre, op=mult)
                nc.vector.tensor_tensor(u2[:], sph, sim, op=mult)
                nc.vector.tensor_sub(uu[:], u1[:], u2[:])
                nc.vector.tensor_tensor(v1[:], sph, sre, op=mult)
                nc.vector.tensor_tensor(v2[:], cph, sim, op=mult)
                nc.vector.tensor_add(vv[:], v1[:], v2[:])
                uu_tiles[bi] = uu
                vv_tiles[bi] = vv

            def emit_block_pe(bi):
                f0, nb = BLOCKS[bi]
                uu, vv = uu_tiles[bi], vv_tiles[bi]
                for j in range(nb):
                    nc.tensor.matmul(psy[:], cosab[:], uu[:, j],
                                     start=state["first_c"], stop=False)
                    state["first_c"] = False
                for j in range(nb):
                    nc.tensor.matmul(psy[:], nsinab[:], vv[:, j],
                                     start=False, stop=False)

            pe_done = set()

            # k=4096 side path first: its matmuls only need x+b12, so they
            # fill the window while the first ah tile is still streaming in
            psx1 = pspool.tile([128, T], f32, tag="ps", name="psx1")
            psx2 = pspool.tile([128, T], f32, tag="ps", name="psx2")
            for ck in range(4):
                for psx, mi in ((psx1, 0), (psx2, 1)):
                    for c0, cn in ((0, 512), (512, T - 512)):
                        nc.tensor.matmul(
                            psx[0:1, c0:c0 + cn],
                            b12_sb[:, ck, mi:mi + 1],
                            xh_sb[:, ck, c0:c0 + cn],
                            start=(ck == 0),
                            stop=(ck == 3),
                        )
            sp1 = scr.tile([1, T], f32, tag="sp1")
            nc.scalar.activation(sp1[:], psx1[0:1, :], Act.Copy)
            qs = scr.tile([1, T], f32, tag="qs")
            nc.vector.tensor_tensor(qs[:], psx2[0:1, :], sp1[:], op=mult)
            f1s = scr.tile([1, 4, 98], f32, tag="f1s")
            qsv = qs[:].rearrange("p (b h t) -> p b h t", b=BPC, h=2)
            nc.vector.tensor_add(f1s[:], qsv[:, :, 0], qsv[:, :, 1])
            s4 = scr.tile([1, 4], f32, tag="s4")
            nc.vector.reduce_sum(out=s4[:], in_=f1s[:],
                                 axis=mybir.AxisListType.X)
            r4 = scr.tile([1, 4, 64], bf16, tag="r4")
            nc.vector.tensor_tensor(
                r4[:],
                s4[:][:, :, None].broadcast_to([1, 4, 64]),
                par_sb[:][:, None, :].broadcast_to([1, 4, 64]),
                op=mult,
            )

            # ---- main loop over frequency tiles ----
            for ft in range(FT):
                # stage-C DVE prep for the block ending at ft-1 (one-ft lag so
                # its sem wait never blocks the Act copy FIFO)
                for bi, (f0, nb) in enumerate(BLOCKS):
                    if f0 + nb == ft:
                        emit_block_dve(bi)
                for bi, (f0, nb) in enumerate(BLOCKS):
                    if bi not in pe_done and ft >= f0 + nb + 2:
                        emit_block_pe(bi)
                        pe_done.add(bi)

                prefetch_ah(ft + 3)
                ah_t = ah_tiles.pop(ft)

                # stage A: projections, one Act copy per m to SBUF bf16
                pc01 = scr.tile([128, 2, T], bf16, tag="pc01",
                                name=f"pc01_{ft}")
                pc23 = scr.tile([128, 2, T], bf16, tag="pc23",
                                name=f"pc23_{ft}")
                pcs = (pc01[:, 0], pc01[:, 1], pc23[:, 0], pc23[:, 1])
                for m in range(4):
                    ps_m = pspool.tile([128, T], f32, tag="ps",
                                       name=f"ps{m}_{ft}")
                    msl = slice(m * 128, (m + 1) * 128)
                    for c0, cn in ((0, 512), (512, T - 512)):
                        for ck in range(4):
                            nc.tensor.matmul(
                                ps_m[:, c0:c0 + cn],
                                ah_t[:, ck, msl],
                                xh_sb[:, ck, c0:c0 + cn],
                                start=(ck == 0),
                                stop=(ck == 3),
                            )
                    nc.scalar.activation(pcs[m], ps_m[:], Act.Copy)

                # stage B: products (one broadcast-AP op), complex combine,
                # one fold over t, reduce
                q = scr.tile([128, 2, 2, T], bf16, tag="q", name=f"q_{ft}")
                nc.vector.tensor_tensor(
                    q[:],
                    pc01[:, :, None, :].broadcast_to([128, 2, 2, T]),
                    pc23[:, None, :, :].broadcast_to([128, 2, 2, T]),
                    op=mult,
                )
                de = scr.tile([128, 2, T], bf16, tag="de", name=f"de_{ft}")
                nc.vector.tensor_sub(de[:, 0], q[:, 0, 0], q[:, 1, 1])
                nc.vector.tensor_add(de[:, 1], q[:, 0, 1], q[:, 1, 0])
                f1 = scr.tile([128, 2, 4, 98], bf16, tag="f1",
                              name=f"f1_{ft}")
                dev = de[:].rearrange("p e (b h t) -> p e b h t", b=BPC, h=2)
                nc.vector.tensor_add(f1[:], dev[:, :, :, 0], dev[:, :, :, 1])
                nc.vector.reduce_sum(
                    out=sresim[:, ft],
                    in_=f1[:],
                    axis=mybir.AxisListType.X,
                )

                if ft == 2:
                    nc.scalar.activation(cphib[:], cphi_sb[:], Act.Copy)
                    nc.scalar.activation(sphib[:], sphi_sb[:], Act.Copy)
                    nc.scalar.activation(cosab[:], cosa_sb[:], Act.Copy)
                    nc.scalar.activation(nsinab[:], nsina_sb[:], Act.Copy)

            for bi, (f0, nb) in enumerate(BLOCKS):
                if f0 + nb >= FT:
                    emit_block_dve(bi)
            for bi in range(len(BLOCKS)):
                if bi not in pe_done:
                    emit_block_pe(bi)
                    pe_done.add(bi)

            # rank-1 k=4096 correction: psy[q, b, s] += 1 * (S4096[b]*par[s]);
            # closes the psy accumulation group
            nc.tensor.matmul(psy[:], onecol[0:1, :], r4[:],
                             start=False, stop=True)

            # ---- stage D: signed sqrt, per-batch l2 norm, store ----
            absy = scr.tile([128, BPC * 64], bf16, tag="absy")
            nc.scalar.activation(absy[:], psy[:], Act.Abs)
            psn = pspool.tile([128, T], f32, tag="ps", name="psn")
            nc.tensor.matmul(psn[0:1, 0:BPC * 64], ones_bf[:], absy[:],
                             start=True, stop=True)
            sqy = scr.tile([128, BPC * 64], bf16, tag="sqy")
            nc.scalar.activation(sqy[:], absy[:], Act.Sqrt)
            sgn = scr.tile([128, BPC * 64], bf16, tag="sgn")
            nc.scalar.activation(sgn[:], psy[:], Act.Sign)
            nsq = scr.tile([1, BPC], f32, tag="nsq")
            nc.vector.reduce_sum(
                out=nsq[:],
                in_=psn[0:1, 0:BPC * 64].rearrange("p (b s) -> p b s", b=BPC),
                axis=mybir.AxisListType.X,
            )
            nc.vector.tensor_scalar_max(nsq[:], nsq[:], 1e-10)
            ys = scr.tile([128, BPC * 64], bf16, tag="ys")
            nc.vector.tensor_mul(ys[:], sqy[:], sgn[:])
            sqn = scr.tile([1, BPC], f32, tag="sqn")
            nc.scalar.activation(sqn[:], nsq[:], Act.Sqrt)
            invn = scr.tile([1, BPC], f32, tag="invn")
            nc.vector.reciprocal(invn[:], sqn[:])

            onesrow = const.tile([1, 128], f32)
            nc.vector.memset(onesrow[:], 1.0)
            psb = pspool.tile([128, T], f32, tag="ps", name="psb")
            nc.tensor.matmul(psb[:, 0:BPC], onesrow[0:1, :], invn[0:1, :],
                             start=True, stop=True)
            inv_b = psb[:, 0:BPC][:, :, None].broadcast_to([128, BPC, 64])
            fin = scr.tile([128, BPC * 64], f32, tag="fin")
            nc.vector.tensor_tensor(
                fin[:].rearrange("p (b s) -> p b s", b=BPC),
                ys[:].rearrange("p (b s) -> p b s", b=BPC),
                inv_b,
                op=mult,
            )
            nc.sync.dma_start(
                y_d.rearrange("b (q s) -> q b s", q=128),
                fin[:].rearrange("p (b s) -> p b s", b=BPC),
            )

    nc.compile()
    return nc


def _round_fp32r(f):
    """RNE to 11 mantissa bits — matches TRN2 fp32r rounding exactly."""
    u = np.ascontiguousarray(f).view(np.uint32)
    drop = 12
    r = u + np.uint32((1 << (drop - 1)) - 1) + ((u >> drop) & np.uint32(1))
    r = (r >> drop) << drop
    return r.view(np.float32)


def _host_prep(x, M1, M2):
    x = np.ascontiguousarray(np.asarray(x, np.float32))
    M1 = np.asarray(M1, np.float32)
    M2 = np.asarray(M2, np.float32)

    h1 = np.argmax(np.abs(M1), axis=1)
    s1 = M1[np.arange(C), h1].astype(np.float64)
    h2 = np.argmax(np.abs(M2), axis=1)
    s2 = M2[np.arange(C), h2].astype(np.float64)

    k = np.arange(NSLOT, dtype=np.float64)
    ang1 = 2 * np.pi * np.outer(h1.astype(np.float64), k) / P
    ang2 = 2 * np.pi * np.outer(h2.astype(np.float64), k) / P
    # a[ft, c, m*128 + j]: m in (A1re, A1im, A2re, A2im), freq = ft*128 + j
    a = np.empty((FT, C, 512), np.float32)
    a1re = (s1[:, None] * np.cos(ang1)).astype(np.float32)
    a1im = (-s1[:, None] * np.sin(ang1)).astype(np.float32)
    a2re = (s2[:, None] * np.cos(ang2)).astype(np.float32)
    a2im = (-s2[:, None] * np.sin(ang2)).astype(np.float32)
    for ft in range(FT):
        ksl = slice(ft * 128, (ft + 1) * 128)
        a[ft, :, 0:128] = a1re[:, ksl]
        a[ft, :, 128:256] = a1im[:, ksl]
        a[ft, :, 256:384] = a2re[:, ksl]
        a[ft, :, 384:512] = a2im[:, ksl]

    # k = 4096: A[c] = s * cos(pi*h) = s * (-1)^h (imag part exactly 0)
    b12 = np.stack([
        (s1 * np.cos(np.pi * h1.astype(np.float64))).astype(np.float32),
        (s2 * np.cos(np.pi * h2.astype(np.float64))).astype(np.float32),
    ], axis=1)
    # y[64q+s] += (1/P) * S4096 * (-1)^s
    par = ((1.0 / P) * np.cos(np.pi * np.arange(64, dtype=np.float64))
           ).astype(np.float32).reshape(1, 64)

    w = np.full(NSLOT, 2.0 / P)
    w[0] = 1.0 / P
    s_idx = np.arange(64, dtype=np.float64)
    phi = 2 * np.pi * np.outer(k, s_idx) / P
    cphi = (w[:, None] * np.cos(phi)).astype(np.float32).reshape(FT, 128, 64)
    sphi = (w[:, None] * np.sin(phi)).astype(np.float32).reshape(FT, 128, 64)

    km = np.arange(128, dtype=np.float64)
    alpha = 2 * np.pi * np.outer(km, km) / 128
    cosa = np.cos(alpha).astype(np.float32)
    nsina = (-np.sin(alpha)).astype(np.float32)

    import ml_dtypes

    xt = np.ascontiguousarray(x.reshape(B * HW, C).T)  # [C, 6272]

    ah = a.astype(ml_dtypes.bfloat16)
    xh = xt.astype(ml_dtypes.bfloat16)
    b12 = b12.astype(ml_dtypes.bfloat16)
    return ah, b12, par, cphi, sphi, cosa, nsina, xh


def _make_in_maps(x, M1, M2):
    ah, b12, par, cphi, sphi, cosa, nsina, xh = _host_prep(x, M1, M2)
    in_maps = []
    for r in range(NCORES):
        in_maps.append({
            "ah": ah,
            "xh": np.ascontiguousarray(xh[:, r * T:(r + 1) * T]),
            "b12": b12,
            "par": par,
            "cphi": cphi,
            "sphi": sphi,
            "cosa": cosa,
            "nsina": nsina,
        })
    return in_maps


def kernel(x, M1, M2):
    from concourse.bass_utils import run_bass_kernel_spmd

    if "nc" not in _CACHE:
        _CACHE["nc"] = _build_program()
    nc = _CACHE["nc"]

    in_maps = _make_in_maps(x, M1, M2)
    res = run_bass_kernel_spmd(nc, in_maps, core_ids=list(range(NCORES)))
    out = np.concatenate([res.results[r]["y"] for r in range(NCORES)], axis=0)
    return out.astype(np.float32)


# revision 29
# speedup vs baseline: 1.2751x; 1.0054x over previous
"""Trainium2 kernel for CompactBilinearLayer (count-sketch bilinear pooling).

Math: reference computes y = l2norm(signed_sqrt(sum_hw Re IFFT(FFT(x@M1)*FFT(x@M2)))).
Since M1/M2 are count-sketch matrices (one +-1 per row), FFT(x@M1) == x @ A1 with
A1[c,k] = s1[c] * exp(-2pi i h1[c] k / P) — a dense [512, K] matrix computable on the
host from M1 in O(C*K). The IFFT is linear, so the spatial sum moves before it.
Hermitian symmetry means only k = 0..4096 are needed; k=4096 has exactly-zero
imaginary parts (sin(pi*h)=0) and is handled by a tiny 1-partition side path plus a
rank-1 (-1)^s correction matmul into the IFFT accumulator.  Per core (4 batch
elements, 784 spatial positions — fully batch-local, no collectives):
  A: P projections = A^T @ x^T, single bf16 matmul per tile — PE-bound at the
     matmul-streaming roofline (the later product stage is bf16 anyway, so
     bf16 operands cost almost no extra error but enable fast weight loads)
  B: S[k,b] = sum_t (P1*P2) per batch: Act stages PSUM->SBUF bf16, DVE does one
     broadcast-AP product op (2x rate), complex combine (D/E), one fold over t,
     then a reduce — bf16 2x where alignment allows
  C: IFFT via two-step factorization n=64q+s, batched every 4 freq tiles:
     Act broadcasts S over s, DVE builds U/V at bf16 2x, PE contracts over k%128
     with bf16 cosa/nsina stationaries into a persistent PSUM accumulator
  D: signed sqrt + per-batch L2 norm + store
"""
import numpy as np

P = 8192
C = 512
FT = 32            # frequency tiles of 128 -> k = 0..4095; k=4096 special-cased
NSLOT = FT * 128
NCORES = 8
BPC = 4            # batch elems per core
HW = 196           # spatial positions per batch elem
T = BPC * HW       # 784 positions per core
B = 32

# stage-C blocks: (first freq tile, number of tiles); small final blocks so the
# tail (last block's DVE prep -> PE -> stage D) is short
BLOCKS = [(0, 4), (4, 4), (8, 4), (12, 4), (16, 4), (20, 4), (24, 4),
          (28, 2), (30, 1), (31, 1)]

_CACHE = {}


def _build_program():
    import concourse.bass as bass
    import concourse.tile as tile
    from concourse import bacc, mybir

    f32 = mybir.dt.float32
    f32r = mybir.dt.float32r
    bf16 = mybir.dt.bfloat16
    nc = bacc.Bacc("TRN2", target_bir_lowering=False, debug=False,
                   num_devices=NCORES)

    ah_d = nc.dram_tensor("ah", [FT, C, 512], bf16, kind="ExternalInput").ap()
    xh_d = nc.dram_tensor("xh", [C, T], bf16, kind="ExternalInput").ap()
    b12_d = nc.dram_tensor("b12", [C, 2], bf16, kind="ExternalInput").ap()
    par_d = nc.dram_tensor("par", [1, 64], f32, kind="ExternalInput").ap()
    cphi_d = nc.dram_tensor("cphi", [FT, 128, 64], f32, kind="ExternalInput").ap()
    sphi_d = nc.dram_tensor("sphi", [FT, 128, 64], f32, kind="ExternalInput").ap()
    cosa_d = nc.dram_tensor("cosa", [128, 128], f32, kind="ExternalInput").ap()
    nsina_d = nc.dram_tensor("nsina", [128, 128], f32, kind="ExternalInput").ap()
    y_d = nc.dram_tensor("y", [BPC, P], f32, kind="ExternalOutput").ap()

    mult = mybir.AluOpType.mult
    Act = mybir.ActivationFunctionType

    with tile.TileContext(nc) as tc:
        with (
            tc.tile_pool(name="const", bufs=1) as const,
            tc.tile_pool(name="apool", bufs=4) as apool,
            tc.tile_pool(name="ps", bufs=3, space="PSUM") as pspool,
            tc.tile_pool(name="py", bufs=1, space="PSUM") as pypool,
            tc.tile_pool(name="scr", bufs=2) as scr,
            tc.tile_pool(name="uv", bufs=2) as uvpool,
        ):
            ah_tiles = {}

            def prefetch_ah(ft, split=False):
                if ft < FT and ft not in ah_tiles:
                    t_ = apool.tile([128, 4, 512], bf16, tag="ah",
                                    name=f"ah_{ft}")
                    if split:
                        for ck in range(4):
                            nc.sync.dma_start(
                                t_[:, ck],
                                ah_d[ft, ck * 128:(ck + 1) * 128, :])
                    else:
                        nc.sync.dma_start(
                            t_[:],
                            ah_d[ft].rearrange("(ck p) m -> p ck m", p=128))
                    ah_tiles[ft] = t_

            # x chunks + b12 first (they gate the k=4096 warm-up matmuls),
            # then the ah stream; stage-C tables ride the gpsimd DMA queue so
            # they never stall the ah prefetch pipeline
            xh_sb = const.tile([128, 4, T], bf16)
            b12_sb = const.tile([128, 4, 2], bf16)
            nc.sync.dma_start(b12_sb[:],
                              b12_d.rearrange("(ck p) m -> p ck m", p=128))
            for ck in range(4):
                nc.sync.dma_start(xh_sb[:, ck],
                                  xh_d[ck * 128:(ck + 1) * 128, :])
            prefetch_ah(0, split=True)
            prefetch_ah(1)
            prefetch_ah(2)
            prefetch_ah(3)

            par_sb = const.tile([1, 64], f32)
            nc.sync.dma_start(par_sb[:], par_d)
            cphi_sb = const.tile([128, FT, 64], f32)
            nc.sync.dma_start(cphi_sb[:], cphi_d.rearrange("kt p s -> p kt s"))
            sphi_sb = const.tile([128, FT, 64], f32)
            nc.sync.dma_start(sphi_sb[:], sphi_d.rearrange("kt p s -> p kt s"))
            cosa_sb = const.tile([128, 128], f32)
            nc.sync.dma_start(cosa_sb[:], cosa_d)
            nsina_sb = const.tile([128, 128], f32)
            nc.sync.dma_start(nsina_sb[:], nsina_d)
            ones_bf = const.tile([128, 1], bf16)
            nc.vector.memset(ones_bf[:], 1.0)
            onecol = const.tile([1, 128], bf16)
            nc.vector.memset(onecol[:], 1.0)

            # preload every activation table set used later (Sqrt lives in its
            # own set — loading it now keeps the ~1.3us table DMA off the tail)
            warm = const.tile([1, 1], f32)
            nc.vector.memset(warm[:], 1.0)
            for fn in (Act.Copy, Act.Abs, Act.Sign, Act.Sqrt):
                nc.scalar.activation(warm[:], warm[:], fn)

            # bf16 copies of the stage-C tables (DVE 2x mode needs bf16);
            # converted at the end of iteration 0 so their DMA wait never
            # blocks the hot Act copy FIFO
            cphib = const.tile([128, FT, 64], bf16)
            sphib = const.tile([128, FT, 64], bf16)
            cosab = const.tile([128, 128], bf16)
            nsinab = const.tile([128, 128], bf16)

            sresim = const.tile([128, FT, 2, 4], f32)
            psy = pypool.tile([128, BPC * 64], f32, tag="py")

            uu_tiles = {}
            vv_tiles = {}
            state = {"first_c": True}

            def emit_block_dve(bi):
                f0, nb = BLOCKS[bi]
                sx = scr.tile([128, nb, 2, 4, 64], bf16, tag=f"sx{nb}_{bi % 2}",
                              name=f"sx_{bi}")
                nc.scalar.activation(
                    sx[:],
                    sresim[:, f0:f0 + nb][:, :, :, :, None].broadcast_to(
                        [128, nb, 2, 4, 64]),
                    Act.Copy,
                )
                cph = cphib[:, f0:f0 + nb, None, :].broadcast_to(
                    [128, nb, 4, 64])
                sph = sphib[:, f0:f0 + nb, None, :].broadcast_to(
                    [128, nb, 4, 64])
                sre = sx[:, :, 0]
                sim = sx[:, :, 1]
                u1 = uvpool.tile([128, nb, 4, 64], bf16, tag=f"u1{nb}",
                                 name=f"u1_{bi}")
                u2 = uvpool.tile([128, nb, 4, 64], bf16, tag=f"u2{nb}",
                                 name=f"u2_{bi}")
                uu = uvpool.tile([128, nb, 4, 64], bf16, tag=f"uu{nb}",
                                 name=f"uu_{bi}")
                v1 = uvpool.tile([128, nb, 4, 64], bf16, tag=f"v1{nb}",
                                 name=f"v1_{bi}")
                v2 = uvpool.tile([128, nb, 4, 64], bf16, tag=f"v2{nb}",
                                 name=f"v2_{bi}")
                vv = uvpool.tile([128, nb, 4, 64], bf16, tag=f"vv{nb}",
                                 name=f"vv_{bi}")
                nc.vector.tensor_tensor(u1[:], cph, sre, op=mult)
                nc.vector.tensor_tensor(u2[:], sph, sim, op=mult)
                nc.vector.tensor_sub(uu[:], u1[:], u2[:])
                nc.vector.tensor_tensor(v1[:], sph, sre, op=mult)
                nc.vector.tensor_tensor(v2[:], cph, sim, op=mult)
                nc.vector.tensor_add(vv[:], v1[:], v2[:])
                uu_tiles[bi] = uu
                vv_tiles[bi] = vv

            def emit_block_pe(bi):
                f0, nb = BLOCKS[bi]
                uu, vv = uu_tiles[bi], vv_tiles[bi]
                for j in range(nb):
                    nc.tensor.matmul(psy[:], cosab[:], uu[:, j],
                                     start=state["first_c"], stop=False)
                    state["first_c"] = False
                for j in range(nb):
                    nc.tensor.matmul(psy[:], nsinab[:], vv[:, j],
                                     start=False, stop=False)

            pe_done = set()

            # k=4096 side path first: its matmuls only need x+b12, so they
            # fill the window while the first ah tile is still streaming in
            psx1 = pspool.tile([128, T], f32, tag="ps", name="psx1")
            psx2 = pspool.tile([128, T], f32, tag="ps", name="psx2")
            for ck in range(4):
                for psx, mi in ((psx1, 0), (psx2, 1)):
                    for c0, cn in ((0, 512), (512, T - 512)):
                        nc.tensor.matmul(
                            psx[0:1, c0:c0 + cn],
                            b12_sb[:, ck, mi:mi + 1],
                            xh_sb[:, ck, c0:c0 + cn],
                            start=(ck == 0),
                            stop=(ck == 3),
                        )
            sp1 = scr.tile([1, T], f32, tag="sp1")
            nc.scalar.activation(sp1[:], psx1[0:1, :], Act.Copy)
            qs = scr.tile([1, T], f32, tag="qs")
            nc.vector.tensor_tensor(qs[:], psx2[0:1, :], sp1[:], op=mult)
            f1s = scr.tile([1, 4, 98], f32, tag="f1s")
            qsv = qs[:].rearrange("p (b h t) -> p b h t", b=BPC, h=2)
            nc.vector.tensor_add(f1s[:], qsv[:, :, 0], qsv[:, :, 1])
            s4 = scr.tile([1, 4], f32, tag="s4")
            nc.vector.reduce_sum(out=s4[:], in_=f1s[:],
                                 axis=mybir.AxisListType.X)
            r4 = scr.tile([1, 4, 64], bf16, tag="r4")
            nc.vector.tensor_tensor(
                r4[:],
                s4[:][:, :, None].broadcast_to([1, 4, 64]),
                par_sb[:][:, None, :].broadcast_to([1, 4, 64]),
                op=mult,
            )

            # ---- main loop over frequency tiles ----
            for ft in range(FT):
                # stage-C DVE prep for the block ending at ft-1 (one-ft lag so
                # its sem wait never blocks the Act copy FIFO)
                for bi, (f0, nb) in enumerate(BLOCKS):
                    if f0 + nb == ft:
                        emit_block_dve(bi)
                for bi, (f0, nb) in enumerate(BLOCKS):
                    if bi not in pe_done and ft >= f0 + nb + 2:
                        emit_block_pe(bi)
                        pe_done.add(bi)

                prefetch_ah(ft + 3)
                ah_t = ah_tiles.pop(ft)

                # stage A: projections, one Act copy per m to SBUF bf16
                pc01 = scr.tile([128, 2, T], bf16, tag="pc01",
                                name=f"pc01_{ft}")
                pc23 = scr.tile([128, 2, T], bf16, tag="pc23",
                                name=f"pc23_{ft}")
                pcs = (pc01[:, 0], pc01[:, 1], pc23[:, 0], pc23[:, 1])
                for m in range(4):
                    ps_m = pspool.tile([128, T], f32, tag="ps",
                                       name=f"ps{m}_{ft}")
                    msl = slice(m * 128, (m + 1) * 128)
                    for c0, cn in ((0, 512), (512, T - 512)):
                        for ck in range(4):
                            nc.tensor.matmul(
                                ps_m[:, c0:c0 + cn],
                                ah_t[:, ck, msl],
                                xh_sb[:, ck, c0:c0 + cn],
                                start=(ck == 0),
                                stop=(ck == 3),
                            )
                    nc.scalar.activation(pcs[m], ps_m[:], Act.Copy)

                # stage B: products (one broadcast-AP op), complex combine,
                # one fold over t, reduce
                q = scr.tile([128, 2, 2, T], bf16, tag="q", name=f"q_{ft}")
                nc.vector.tensor_tensor(
                    q[:],
                    pc01[:, :, None, :].broadcast_to([128, 2, 2, T]),
                    pc23[:, None, :, :].broadcast_to([128, 2, 2, T]),
                    op=mult,
                )
                de = scr.tile([128, 2, T], bf16, tag="de", name=f"de_{ft}")
                nc.vector.tensor_sub(de[:, 0], q[:, 0, 0], q[:, 1, 1])
                nc.vector.tensor_add(de[:, 1], q[:, 0, 1], q[:, 1, 0])
                f1 = scr.tile([128, 2, 4, 98], bf16, tag="f1",
                              name=f"f1_{ft}")
                dev = de[:].rearrange("p e (b h t) -> p e b h t", b=BPC, h=2)
                nc.vector.tensor_add(f1[:], dev[:, :, :, 0], dev[:, :, :, 1])
                nc.vector.reduce_sum(
                    out=sresim[:, ft],
                    in_=f1[:],
                    axis=mybir.AxisListType.X,
                )

                if ft == 2:
                    nc.scalar.activation(cphib[:], cphi_sb[:], Act.Copy)
                    nc.scalar.activation(sphib[:], sphi_sb[:], Act.Copy)
                    nc.scalar.activation(cosab[:], cosa_sb[:], Act.Copy)
                    nc.scalar.activation(nsinab[:], nsina_sb[:], Act.Copy)

            for bi, (f0, nb) in enumerate(BLOCKS):
                if f0 + nb >= FT:
                    emit_block_dve(bi)
            for bi in range(len(BLOCKS)):
                if bi not in pe_done:
                    emit_block_pe(bi)
                    pe_done.add(bi)

            # rank-1 k=4096 correction: psy[q, b, s] += 1 * (S4096[b]*par[s]);
            # closes the psy accumulation group
            nc.tensor.matmul(psy[:], onecol[0:1, :], r4[:],
                             start=False, stop=True)

            # ---- stage D: signed sqrt, per-batch l2 norm, store ----
            absy = scr.tile([128, BPC * 64], bf16, tag="absy")
            nc.scalar.activation(absy[:], psy[:], Act.Abs)
            psn = pspool.tile([128, T], f32, tag="ps", name="psn")
            nc.tensor.matmul(psn[0:1, 0:BPC * 64], ones_bf[:], absy[:],
                             start=True, stop=True)
            sqy = scr.tile([128, BPC * 64], bf16, tag="sqy")
            nc.scalar.activation(sqy[:], absy[:], Act.Sqrt)
            sgn = scr.tile([128, BPC * 64], bf16, tag="sgn")
            nc.scalar.activation(sgn[:], psy[:], Act.Sign)
            nsq = scr.tile([1, BPC], f32, tag="nsq")
            nc.vector.reduce_sum(
                out=nsq[:],
                in_=psn[0:1, 0:BPC * 64].rearrange("p (b s) -> p b s", b=BPC),
                axis=mybir.AxisListType.X,
            )
            nc.vector.tensor_scalar_max(nsq[:], nsq[:], 1e-10)
            ys = scr.tile([128, BPC * 64], bf16, tag="ys")
            nc.vector.tensor_mul(ys[:], sqy[:], sgn[:])
            sqn = scr.tile([1, BPC], f32, tag="sqn")
            nc.scalar.activation(sqn[:], nsq[:], Act.Sqrt)
            invn = scr.tile([1, BPC], f32, tag="invn")
            nc.vector.reciprocal(invn[:], sqn[:])

            onesrow = const.tile([1, 128], f32)
            nc.vector.memset(onesrow[:], 1.0)
            psb = pspool.tile([128, T], f32, tag="ps", name="psb")
            nc.tensor.matmul(psb[:, 0:BPC], onesrow[0:1, :], invn[0:1, :],
                             start=True, stop=True)
            inv_b = psb[:, 0:BPC][:, :, None].broadcast_to([128, BPC, 64])
            fin = scr.tile([128, BPC * 64], f32, tag="fin")
            nc.vector.tensor_tensor(
                fin[:].rearrange("p (b s) -> p b s", b=BPC),
                ys[:].rearrange("p (b s) -> p b s", b=BPC),
                inv_b,
                op=mult,
            )
            nc.sync.dma_start(
                y_d.rearrange("b (q s) -> q b s", q=128),
                fin[:].rearrange("p (b s) -> p b s", b=BPC),
            )

    nc.compile()
    return nc


def _host_prep(x, M1, M2):
    x = np.ascontiguousarray(np.asarray(x, np.float32))
    M1 = np.asarray(M1, np.float32)
    M2 = np.asarray(M2, np.float32)

    h1 = np.argmax(np.abs(M1), axis=1)
    s1 = M1[np.arange(C), h1].astype(np.float64)
    h2 = np.argmax(np.abs(M2), axis=1)
    s2 = M2[np.arange(C), h2].astype(np.float64)

    k = np.arange(NSLOT, dtype=np.float64)
    ang1 = 2 * np.pi * np.outer(h1.astype(np.float64), k) / P
    ang2 = 2 * np.pi * np.outer(h2.astype(np.float64), k) / P
    # a[ft, c, m*128 + j]: m in (A1re, A1im, A2re, A2im), freq = ft*128 + j
    a = np.empty((FT, C, 512), np.float32)
    a1re = (s1[:, None] * np.cos(ang1)).astype(np.float32)
    a1im = (-s1[:, None] * np.sin(ang1)).astype(np.float32)
    a2re = (s2[:, None] * np.cos(ang2)).astype(np.float32)
    a2im = (-s2[:, None] * np.sin(ang2)).astype(np.float32)
    for ft in range(FT):
        ksl = slice(ft * 128, (ft + 1) * 128)
        a[ft, :, 0:128] = a1re[:, ksl]
        a[ft, :, 128:256] = a1im[:, ksl]
        a[ft, :, 256:384] = a2re[:, ksl]
        a[ft, :, 384:512] = a2im[:, ksl]

    # k = 4096: A[c] = s * cos(pi*h) = s * (-1)^h (imag part exactly 0)
    b12 = np.stack([
        (s1 * np.cos(np.pi * h1.astype(np.float64))).astype(np.float32),
        (s2 * np.cos(np.pi * h2.astype(np.float64))).astype(np.float32),
    ], axis=1)
    # y[64q+s] += (1/P) * S4096 * (-1)^s
    par = ((1.0 / P) * np.cos(np.pi * np.arange(64, dtype=np.float64))
           ).astype(np.float32).reshape(1, 64)

    w = np.full(NSLOT, 2.0 / P)
    w[0] = 1.0 / P
    s_idx = np.arange(64, dtype=np.float64)
    phi = 2 * np.pi * np.outer(k, s_idx) / P
    cphi = (w[:, None] * np.cos(phi)).astype(np.float32).reshape(FT, 128, 64)
    sphi = (w[:, None] * np.sin(phi)).astype(np.float32).reshape(FT, 128, 64)

    km = np.arange(128, dtype=np.float64)
    alpha = 2 * np.pi * np.outer(km, km) / 128
    cosa = np.cos(alpha).astype(np.float32)
    nsina = (-np.sin(alpha)).astype(np.float32)

    import ml_dtypes

    xt = np.ascontiguousarray(x.reshape(B * HW, C).T)  # [C, 6272]

    ah = a.astype(ml_dtypes.bfloat16)
    xh = xt.astype(ml_dtypes.bfloat16)
    b12 = b12.astype(ml_dtypes.bfloat16)
    return ah, b12, par, cphi, sphi, cosa, nsina, xh


def _make_in_maps(x, M1, M2):
    ah, b12, par, cphi, sphi, cosa, nsina, xh = _host_prep(x, M1, M2)
    in_maps = []
    for r in range(NCORES):
        in_maps.append({
            "ah": ah,
            "xh": np.ascontiguousarray(xh[:, r * T:(r + 1) * T]),
            "b12": b12,
            "par": par,
            "cphi": cphi,
            "sphi": sphi,
            "cosa": cosa,
            "nsina": nsina,
        })
    return in_maps


def kernel(x, M1, M2):
    from concourse.bass_utils import run_bass_kernel_spmd

    if "nc" not in _CACHE:
        _CACHE["nc"] = _build_program()
    nc = _CACHE["nc"]

    in_maps = _make_in_maps(x, M1, M2)
    res = run_bass_kernel_spmd(nc, in_maps, core_ids=list(range(NCORES)))
    out = np.concatenate([res.results[r]["y"] for r in range(NCORES)], axis=0)
    return out.astype(np.float32)
